# revision 31
# baseline (speedup 1.0000x reference)
"""CenterLoss on Trainium2 (raw Bass, SPMD over 8 NeuronCores).

Computes mean_i ||x_i - centers[label_i]||^2 (the reference clamps each
distance to [1e-12, 1e12], which never binds for this data regime).

Sharding (vocab/class-parallel, per the hint):
  - centers [100000, 512] is split row-wise into 8 shards of 12500 rows.
  - x [256, 512] and the labels are replicated to all cores.
  - Host-side sharding prep: per-core local labels = label - shard_base,
    with out-of-shard labels replaced by a huge sentinel that the
    gather's bounds check drops; a per-core f32 weight vector holds
    1/256 for in-shard rows and 0 otherwise.  Each core produces
    sum(weight_i * dist_i) — its partial of the final mean — and the
    host sums the 8 partial scalars (the unshard step).

Device program per core (identical SPMD image, different data):
  layout: batch row (p*2 + t) lives at partition p, column t (keeps
  every DMA innermost-contiguous; row order is irrelevant to the sum).
    lt [128, 2]     i32   <- local labels   (HWDGE, issued first)
    mt [128, 2]     f32   <- weights        (HWDGE)
    xt [128, 2*512] f32   <- x              (HWDGE, 512 KB)
    gt [128, 2*512] f32   memset 0 (DVE), then ONE indirect SWDGE
                          gather of all 256 rows (offset AP [128,2]):
                          row (p,t) <- centers_shard[lt[p,t]]
  DVE:  gt = xt - gt
  ACT:  per column t: Square activation with accum_out -> rs[:, t]
        (the activation table is prefetched by a dummy square at kernel
        start, overlapping the DMA waits)
  PE :  acc[1,1] += mt[:, t]^T @ rs[:, t]  (t = 0,1; PSUM accumulate).
        Out-of-shard rows have weight 0, so whatever the bounds-checked
        gather leaves there (0 from the memset, or x after the sub)
        contributes nothing; the memset keeps every value finite.
  DVE:  res <- acc (PSUM -> SBUF); HWDGE stores the [1,1] scalar.

Raw Bass (not Tile) because this container's walrus build accepts only
one folded sync-wait per instruction ("Too many sync wait commands") and
rejects the fused DVE tensor_tensor_reduce encoding ("ISA wrong length");
standalone wait_ge instructions and BIR-native ops sidestep both.
"""

import numpy as np

import concourse.bass as bass
from concourse import mybir
from concourse.bass_utils import run_bass_kernel_spmd

NUM_CLASSES = 100000
FEAT = 512
BATCH = 256
N_CORES = 8
ROWS = NUM_CLASSES // N_CORES  # 12500 center rows per core
P = 128
NT = BATCH // P  # 2 columns per partition
OOB_SENTINEL = 2_000_000_000  # > bounds_check, still valid int32

_cache: dict = {}

# test.py reads this after calling kernel() for exec_time_ns / trace.
LAST_RESULTS = None


def _indirect_gather_on_queue(nc, queue, out, in_, offset_ap, bounds_reg):
    """nc.gpsimd.indirect_dma_start with a selectable SWDGE queue.

    The stock helper hardcodes queue="qPoolDynamic"; with
    num_swdge_queues=2 a second queue exists and lets the two gathers'
    descriptor generation run in parallel (it executes queue-side).
    """
    gp = nc.gpsimd
    out_ap = gp.lower_ap_dma(out, for_indirect_dma=True)
    in_ap = gp.lower_ap_dma(in_, for_indirect_dma=True)
    assert len(in_ap) == 1 and len(out_ap) == 1
    off = gp.lower_ap_dma(offset_ap)
    assert len(off) == 1
    in_ap.append(off[0])
    ap_shape = in_.shape
    coef = 1
    for i in range(1, len(ap_shape)):
        coef *= ap_shape[i]
    in_ap[0].dynamic_ap_info = mybir.DynamicAccessPatternInfo(
        c=0,
        actual_ap=out.ap,
        indirect_dim_max_index=ap_shape[0],
        offset_expr=[
            mybir.DynamicAccessPatternOffsetExpr(
                coef=coef,
                aff_expr=mybir.DynamicAccessPatternOffsetExprAffExpr(
                    kind="IndirectArgId", arg_id=1
                ),
            )
        ],
    )
    return gp.add_instruction(
        mybir.InstDMACopy(
            name=nc.get_next_instruction_name(),
            queue=queue,
            mode="Copy",
            ins=in_ap + [gp.lower_val_access(bounds_reg)],
            outs=out_ap,
            oob_is_err=False,
            cce_op=mybir.AluOpType.bypass,
        )
    )


def _build() -> bass.Bass:
    nc = bass.Bass(enable_partition_id=False, num_swdge_queues=2)
    x = nc.dram_tensor("x", [BATCH, FEAT], mybir.dt.float32, kind="ExternalInput")
    lab = nc.dram_tensor("lab", [BATCH], mybir.dt.int32, kind="ExternalInput")
    msk = nc.dram_tensor("msk", [BATCH], mybir.dt.float32, kind="ExternalInput")
    cen = nc.dram_tensor("cen", [ROWS, FEAT], mybir.dt.float32, kind="ExternalInput")
    out = nc.dram_tensor("out", [1, 1], mybir.dt.float32, kind="ExternalOutput")

    # batch row (p*NT + t) -> partition p, column t
    x_v = x.rearrange("(p t) d -> p t d", t=NT)
    lab_v = lab.rearrange("(p t) -> p t", t=NT)
    msk_v = msk.rearrange("(p t) -> p t", t=NT)

    from contextlib import ExitStack

    with ExitStack() as ctx:
        xt = ctx.enter_context(nc.sbuf_tensor([P, NT * FEAT], mybir.dt.float32))
        gt = ctx.enter_context(nc.sbuf_tensor([P, NT * FEAT], mybir.dt.float32))
        gtb = ctx.enter_context(nc.sbuf_tensor([P, NT * FEAT], mybir.dt.bfloat16))
        sq = ctx.enter_context(nc.sbuf_tensor([P, NT * FEAT], mybir.dt.bfloat16))
        lt = ctx.enter_context(nc.sbuf_tensor([P, NT], mybir.dt.int32))
        mt = ctx.enter_context(nc.sbuf_tensor([P, NT], mybir.dt.float32))
        rs = ctx.enter_context(nc.sbuf_tensor([P, NT], mybir.dt.float32))
        res = ctx.enter_context(nc.sbuf_tensor([1, 1], mybir.dt.float32))
        acc = ctx.enter_context(nc.psum_tensor([1, 1], mybir.dt.float32))
        s_x0 = ctx.enter_context(nc.semaphore())   # xt col 0 load done (+16)
        s_x1 = ctx.enter_context(nc.semaphore())   # xt col 1 load done (+16)
        s_l = ctx.enter_context(nc.semaphore())    # lt load done (+16)
        s_m = ctx.enter_context(nc.semaphore())    # mt load done (+16)
        s_ms = ctx.enter_context(nc.semaphore())   # gt memsets done (+1 each)
        s_g0 = ctx.enter_context(nc.semaphore())   # gather col 0 done (+16)
        s_g1 = ctx.enter_context(nc.semaphore())   # gather col 1 done (+16)
        s_v = ctx.enter_context(nc.semaphore())    # DVE subs done (+1 each)
        s_a = ctx.enter_context(nc.semaphore())    # ACT squares done (+1 each)
        s_mm = ctx.enter_context(nc.semaphore())   # PE matmuls done (+1)
        s_res = ctx.enter_context(nc.semaphore())  # res copy done (+1)
        s_out = ctx.enter_context(nc.semaphore())  # final store done (+16)
        block = ctx.enter_context(nc.Block())
        gt3 = gt[:].rearrange("p (t d) -> p t d", t=NT)
        gtb3 = gtb[:].rearrange("p (t d) -> p t d", t=NT)
        sq3 = sq[:].rearrange("p (t d) -> p t d", t=NT)
        xt3 = xt[:].rearrange("p (t d) -> p t d", t=NT)

        @block.sync
        def _(sync: bass.BassEngine):
            # x halves on separate HWDGE queues; column 0 first so the
            # first subtract's input lands earliest.
            sync.dma_start(out=xt3[:, 0, :], in_=x_v[:, 0, :]).then_inc(s_x0, 16)
            sync.dma_start(out=mt[:], in_=msk_v).then_inc(s_m, 16)
            sync.dma_start(out=xt3[:, 1, :], in_=x_v[:, 1, :]).then_inc(s_x1, 16)
            # Folded wait: the store is enqueued now, armed on s_res, and
            # fires the moment the result copy lands (no sequencer lag).
            sync.dma_start(out=out[:], in_=res[:])._wait_ge(s_res, 1).then_inc(
                s_out, 16
            )
            sync.wait_ge(s_out, 16)

        @block.gpsimd
        def _(gpsimd: bass.BassEngine):
            # Bounds register written before the waits so the gathers
            # issue immediately once the label load lands.
            bounds_reg = gpsimd.to_reg(ROWS - 1)
            # Label load rides the same SWDGE queue as gather 0: FIFO
            # order plus the armed semaphore below.
            gpsimd.dma_start(out=lt[:], in_=lab_v).then_inc(s_l, 16)
            # Pool zeroes column 1 while DVE zeroes column 0 (halves the
            # init latency); the sequencer-side wait covers both before
            # the gathers are enqueued (WAW on gt).
            gpsimd.memset(gt3[:, 1, :], 0.0).then_inc(s_ms, 1)
            gpsimd.wait_ge(s_ms, 2)
            # Two 128-index gathers (a single 256-index gather measured
            # ~3x slower end-to-end) on SEPARATE SWDGE queues, each
            # armed in-queue on the label load (one folded wait is the
            # DMA budget) so they fire the instant the labels land;
            # per-gather semaphores since completion order is free.
            for t, sem in ((0, s_g0), (1, s_g1)):
                _indirect_gather_on_queue(
                    nc,
                    "qPoolDynamic" + ("1" if t else ""),
                    gt3[:, t, :],
                    cen[:],
                    lt[:, t : t + 1],
                    bounds_reg,
                )._wait_ge(s_l, 16).then_inc(sem, 16)

        @block.vector
        def _(vector: bass.BassEngine):
            vector.memset(gt3[:, 0, :], 0.0).then_inc(s_ms, 1)
            for t, sem, s_xh in ((0, s_g0, s_x0), (1, s_g1, s_x1)):
                vector.wait_ge(s_xh, 16)
                vector.wait_ge(sem, 16)
                vector.tensor_sub(
                    out=gtb3[:, t, :], in0=xt3[:, t, :], in1=gt3[:, t, :]
                ).then_inc(s_v, 1)
            vector.wait_ge(s_mm, 1)
            vector.tensor_copy(out=res[:], in_=acc[:]).then_inc(s_res, 1)

        @block.scalar
        def _(scalar: bass.BassEngine):
            # Dummy square: prefetches the ACT function table while the
            # DMAs are still in flight (first ACTIVATE triggers the load).
            # Reads the preamble's 0.0 const tile (barrier-synced), never
            # uninitialized SBUF.
            scalar.square(out=res[:], in_=nc.const_aps.tensor(0.0, [1, 1]))
            for t in range(NT):
                scalar.wait_ge(s_v, t + 1)
                scalar.activation(
                    out=sq3[:, t, :],
                    in_=gtb3[:, t, :],
                    func=mybir.ActivationFunctionType.Square,
                    accum_out=rs[:, t : t + 1],
                ).then_inc(s_a, 1)

        @block.tensor
        def _(tensor: bass.BassEngine):
            tensor.wait_ge(s_m, 16)
            tensor.wait_ge(s_a, 2)
            for t in range(NT):
                mm = tensor.matmul(
                    out=acc[:],
                    lhsT=mt[:, t : t + 1],
                    rhs=rs[:, t : t + 1],
                    start=(t == 0),
                    stop=(t == NT - 1),
                )
            mm.then_inc(s_mm, 1)

    return nc


def kernel(x: np.ndarray, label: np.ndarray, centers: np.ndarray) -> np.ndarray:
    global LAST_RESULTS
    x = np.ascontiguousarray(np.asarray(x, dtype=np.float32))
    centers = np.ascontiguousarray(np.asarray(centers, dtype=np.float32))
    lbl = np.asarray(label).astype(np.int64).ravel()
    assert x.shape == (BATCH, FEAT), x.shape
    assert centers.shape == (NUM_CLASSES, FEAT), centers.shape
    assert lbl.shape == (BATCH,), lbl.shape

    in_maps = []
    for i in range(N_CORES):
        loc = lbl - i * ROWS
        valid = (loc >= 0) & (loc < ROWS)
        loc32 = np.where(valid, loc, OOB_SENTINEL).astype(np.int32)
        wt = valid.astype(np.float32) / np.float32(BATCH)
        in_maps.append(
            {
                "x": x,
                "lab": loc32,
                "msk": wt,
                "cen": centers[i * ROWS : (i + 1) * ROWS],
            }
        )

    if "nc" not in _cache:
        _cache["nc"] = _build()
    res = run_bass_kernel_spmd(_cache["nc"], in_maps, core_ids=list(range(N_CORES)))
    LAST_RESULTS = res

    total = np.float64(0.0)
    for r in res.results:
        total += np.float64(r["out"][0, 0])
    return np.float32(total)


# revision 32
# speedup vs baseline: 1.2966x; 1.2966x over previous
"""CenterLoss on Trainium2 (raw Bass, SPMD over 8 NeuronCores).

Computes mean_i ||x_i - centers[label_i]||^2 (the reference clamps each
distance to [1e-12, 1e12], which never binds for this data regime).

Sharding (data-parallel over the batch, per the hint's second option:
"replicate centers"):
  - x [256, 512] and label [256] are split into 8 shards of 32 rows.
  - centers [100000, 512] is replicated to every core; each core only
    TOUCHES its 32 gathered rows (~64 KB) in HBM, so the kernel stays in
    the memory-latency regime.  Compared to vocab-sharding this cuts the
    per-core indirect-gather descriptor count from 256 to 32 (descriptor
    generation is the dominant serial cost at ~9 ns/descriptor) and
    needs no label localization, masks, bounds sentinels, or tile
    zero-fill.
  - Each core returns sum(dist_rows)/256 — its partial of the final
    mean — and the host sums the 8 partial scalars (the unshard step).

Device program per core (identical SPMD image, different data):
    lt [32, 1]   i32 <- label shard   (HWDGE, issued first)
    xt [32, 512] f32 <- x shard       (HWDGE, 64 KB)
    gt [32, 512] f32 <- ONE indirect SWDGE gather (32 descriptors),
                        enqueued at kernel start and ARMED in-queue on
                        the label-load semaphore (a DMA instruction's
                        one folded wait), so it fires the instant the
                        labels land with no sequencer round-trip.
  DVE:  gt = xt - gt
  ACT:  Square activation with scale=1/16 and accum_out:
        rs[p] = sum_d ((x-c)[p,d]/16)^2 = dist_p/256.  (The activation
        table is prefetched by a dummy square on the preamble's 0.0
        const tile, overlapping the DMA waits.)
  PE :  acc[1,1] = ones[32]^T @ rs  (ones from the preamble const pool)
  DVE:  res <- acc (PSUM -> SBUF); the HWDGE store was enqueued at
        kernel start armed on the copy's semaphore.

Raw Bass (not Tile) because this container's walrus build accepts only
one folded sync-wait per instruction ("Too many sync wait commands") and
rejects the fused DVE tensor_tensor_reduce encoding ("ISA wrong length");
standalone wait_ge instructions and BIR-native ops sidestep both.
"""

import numpy as np

import concourse.bass as bass
from concourse import mybir
from concourse.bass_utils import run_bass_kernel_spmd

NUM_CLASSES = 100000
FEAT = 512
BATCH = 256
N_CORES = 8
SHARD = BATCH // N_CORES  # 32 batch rows per core

_cache: dict = {}

# test.py reads this after calling kernel() for exec_time_ns / trace.
LAST_RESULTS = None


def _build() -> bass.Bass:
    nc = bass.Bass(enable_partition_id=False)
    x = nc.dram_tensor("x", [SHARD, FEAT], mybir.dt.float32, kind="ExternalInput")
    lab = nc.dram_tensor("lab", [SHARD], mybir.dt.int32, kind="ExternalInput")
    cen = nc.dram_tensor(
        "cen", [NUM_CLASSES, FEAT], mybir.dt.float32, kind="ExternalInput"
    )
    out = nc.dram_tensor("out", [1, 1], mybir.dt.float32, kind="ExternalOutput")

    with (
        nc.sbuf_tensor([SHARD, FEAT], mybir.dt.float32) as xt,
        nc.sbuf_tensor([SHARD, FEAT], mybir.dt.float32) as gt,
        nc.sbuf_tensor([SHARD, FEAT], mybir.dt.float32) as sq,
        nc.sbuf_tensor([SHARD, 1], mybir.dt.int32) as lt,
        nc.sbuf_tensor([SHARD, 1], mybir.dt.float32) as rs,
        nc.sbuf_tensor([1, 1], mybir.dt.float32) as res,
        nc.psum_tensor([1, 1], mybir.dt.float32) as acc,
        nc.semaphore() as s_l,    # lt load done (+16)
        nc.semaphore() as s_x,    # xt load done (+16)
        nc.semaphore() as s_g,    # gather done (+16)
        nc.semaphore() as s_v,    # DVE sub done (+1)
        nc.semaphore() as s_a,    # ACT square done (+1)
        nc.semaphore() as s_mm,   # PE matmul done (+1)
        nc.semaphore() as s_res,  # res copy done (+1)
        nc.semaphore() as s_out,  # final store done (+16)
        nc.Block() as block,
    ):

        @block.sync
        def _(sync: bass.BassEngine):
            sync.dma_start(out=lt[:], in_=lab[:, None]).then_inc(s_l, 16)
            sync.dma_start(out=xt[:], in_=x[:, :]).then_inc(s_x, 16)
            # Folded wait: the store is enqueued now, armed on s_res, and
            # fires the moment the result copy lands (no sequencer lag).
            sync.dma_start(out=out[:], in_=res[:])._wait_ge(s_res, 1).then_inc(
                s_out, 16
            )
            sync.wait_ge(s_out, 16)

        @block.gpsimd
        def _(gpsimd: bass.BassEngine):
            # Enqueued immediately; the queue holds it armed on the label
            # load (the one folded wait a DMA instruction may carry).
            # All labels are in [0, NUM_CLASSES) by the input contract;
            # the bounds check only guards against garbage indices.
            gpsimd.indirect_dma_start(
                out=gt[:],
                out_offset=None,
                in_=cen[:],
                in_offset=bass.IndirectOffsetOnAxis(ap=lt[:, :1], axis=0),
                bounds_check=NUM_CLASSES - 1,
                oob_is_err=False,
            )._wait_ge(s_l, 16).then_inc(s_g, 16)

        @block.vector
        def _(vector: bass.BassEngine):
            vector.wait_ge(s_x, 16)
            vector.wait_ge(s_g, 16)
            vector.tensor_sub(out=gt[:], in0=xt[:], in1=gt[:]).then_inc(s_v, 1)
            vector.wait_ge(s_mm, 1)
            vector.tensor_copy(out=res[:], in_=acc[:]).then_inc(s_res, 1)

        @block.scalar
        def _(scalar: bass.BassEngine):
            # Dummy square: prefetches the ACT function table while the
            # DMAs are still in flight (first ACTIVATE triggers the
            # load).  Reads the preamble's barrier-synced 0.0 const.
            scalar.square(out=res[:], in_=nc.const_aps.tensor(0.0, [1, 1]))
            scalar.wait_ge(s_v, 1)
            # rs[p] = sum_d ((x-c)/16)^2 = dist_p / 256
            scalar.activation(
                out=sq[:],
                in_=gt[:],
                func=mybir.ActivationFunctionType.Square,
                scale=1.0 / 16.0,
                accum_out=rs[:, :1],
            ).then_inc(s_a, 1)

        @block.tensor
        def _(tensor: bass.BassEngine):
            tensor.wait_ge(s_a, 1)
            tensor.matmul(
                out=acc[:],
                lhsT=nc.const_aps.tensor(1.0, [SHARD, 1]),
                rhs=rs[:, :1],
                start=True,
                stop=True,
            ).then_inc(s_mm, 1)

    return nc


def kernel(x: np.ndarray, label: np.ndarray, centers: np.ndarray) -> np.ndarray:
    global LAST_RESULTS
    x = np.ascontiguousarray(np.asarray(x, dtype=np.float32))
    centers = np.ascontiguousarray(np.asarray(centers, dtype=np.float32))
    lbl = np.asarray(label).astype(np.int64).ravel()
    assert x.shape == (BATCH, FEAT), x.shape
    assert centers.shape == (NUM_CLASSES, FEAT), centers.shape
    assert lbl.shape == (BATCH,), lbl.shape
    lbl32 = lbl.astype(np.int32)

    in_maps = []
    for i in range(N_CORES):
        sl = slice(i * SHARD, (i + 1) * SHARD)
        in_maps.append({"x": x[sl], "lab": lbl32[sl], "cen": centers})

    if "nc" not in _cache:
        _cache["nc"] = _build()
    res = run_bass_kernel_spmd(_cache["nc"], in_maps, core_ids=list(range(N_CORES)))
    LAST_RESULTS = res

    total = np.float64(0.0)
    for r in res.results:
        total += np.float64(r["out"][0, 0])
    return np.float32(total)


# revision 33
# speedup vs baseline: 1.3349x; 1.0295x over previous
"""CenterLoss on Trainium2 (raw Bass, SPMD over 8 NeuronCores).

Computes mean_i ||x_i - centers[label_i]||^2 (the reference clamps each
distance to [1e-12, 1e12], which never binds for this data regime).

Sharding (data-parallel over the batch, per the hint's second option:
"replicate centers"):
  - x [256, 512] and label [256] are split into 8 shards of 32 rows.
  - centers [100000, 512] is replicated to every core; each core only
    TOUCHES its 32 gathered rows (~64 KB) in HBM, so the kernel stays in
    the memory-latency regime.  Compared to vocab-sharding this cuts the
    per-core indirect-gather descriptor count from 256 to 32 (descriptor
    generation is the dominant serial cost at ~9 ns/descriptor) and
    needs no label localization, masks, bounds sentinels, or tile
    zero-fill.
  - Each core returns sum(dist_rows)/256 — its partial of the final
    mean — and the host sums the 8 partial scalars (the unshard step).

Device program per core (identical SPMD image, different data):
    lt [32, 1]   i32 <- label shard   (HWDGE, issued first)
    xt [32, 512] f32 <- x shard       (HWDGE, 64 KB)
    gt [32, 512] f32 <- ONE indirect SWDGE gather (32 descriptors),
                        enqueued at kernel start and ARMED in-queue on
                        the label-load semaphore (a DMA instruction's
                        one folded wait), so it fires the instant the
                        labels land with no sequencer round-trip.
  DVE:  gt = xt - gt
  ACT:  Square activation with scale=1/16 and accum_out:
        rs[p] = sum_d ((x-c)[p,d]/16)^2 = dist_p/256.  (The activation
        table is prefetched by a dummy square on the preamble's 0.0
        const tile, overlapping the DMA waits.)
  PE :  acc[1,1] = ones[32]^T @ rs  (ones from the preamble const pool)
  DVE:  res <- acc (PSUM -> SBUF); the HWDGE store was enqueued at
        kernel start armed on the copy's semaphore.

Raw Bass (not Tile) because this container's walrus build accepts only
one folded sync-wait per instruction ("Too many sync wait commands") and
rejects the fused DVE tensor_tensor_reduce encoding ("ISA wrong length");
standalone wait_ge instructions and BIR-native ops sidestep both.
"""

import numpy as np

import concourse.bass as bass
from concourse import mybir
from concourse.bass_utils import run_bass_kernel_spmd

NUM_CLASSES = 100000
FEAT = 512
BATCH = 256
N_CORES = 8
SHARD = BATCH // N_CORES  # 32 batch rows per core

_cache: dict = {}

# test.py reads this after calling kernel() for exec_time_ns / trace.
LAST_RESULTS = None


def _build() -> bass.Bass:
    nc = bass.Bass(enable_partition_id=False)
    x = nc.dram_tensor("x", [SHARD, FEAT], mybir.dt.float32, kind="ExternalInput")
    lab = nc.dram_tensor("lab", [SHARD], mybir.dt.int32, kind="ExternalInput")
    cen = nc.dram_tensor(
        "cen", [NUM_CLASSES, FEAT], mybir.dt.float32, kind="ExternalInput"
    )
    out = nc.dram_tensor("out", [1, 1], mybir.dt.float32, kind="ExternalOutput")

    with (
        nc.sbuf_tensor([SHARD, FEAT], mybir.dt.float32) as xt,
        nc.sbuf_tensor([SHARD, FEAT], mybir.dt.float32) as gt,
        nc.sbuf_tensor([SHARD, FEAT], mybir.dt.float32) as sq,
        nc.sbuf_tensor([SHARD, 1], mybir.dt.int32) as lt,
        nc.sbuf_tensor([SHARD, 1], mybir.dt.float32) as rs,
        nc.sbuf_tensor([1, 1], mybir.dt.float32) as res,
        nc.psum_tensor([1, 1], mybir.dt.float32) as acc,
        nc.semaphore() as s_l,    # lt load done (+16)
        nc.semaphore() as s_x,    # xt load done (+16)
        nc.semaphore() as s_g,    # gather done (+16)
        nc.semaphore() as s_v,    # DVE sub done (+1)
        nc.semaphore() as s_a,    # ACT square done (+1)
        nc.semaphore() as s_mm,   # PE matmul done (+1)
        nc.semaphore() as s_res,  # res copy done (+1)
        nc.semaphore() as s_out,  # final store done (+16)
        nc.Block() as block,
    ):

        @block.sync
        def _(sync: bass.BassEngine):
            sync.dma_start(out=lt[:], in_=lab[:, None], single_packet=True).then_inc(
                s_l, 16
            )
            sync.dma_start(out=xt[:], in_=x[:, :]).then_inc(s_x, 16)
            # Folded wait: the store is enqueued now, armed on s_res, and
            # fires the moment the result copy lands (no sequencer lag).
            sync.dma_start(out=out[:], in_=res[:])._wait_ge(s_res, 1).then_inc(
                s_out, 16
            )
            sync.wait_ge(s_out, 16)

        @block.gpsimd
        def _(gpsimd: bass.BassEngine):
            # Enqueued immediately; the queue holds it armed on the label
            # load (the one folded wait a DMA instruction may carry).
            # All labels are in [0, NUM_CLASSES) by the input contract;
            # the bounds check only guards against garbage indices.
            gpsimd.indirect_dma_start(
                out=gt[:],
                out_offset=None,
                in_=cen[:],
                in_offset=bass.IndirectOffsetOnAxis(ap=lt[:, :1], axis=0),
                bounds_check=NUM_CLASSES - 1,
                oob_is_err=False,
            )._wait_ge(s_l, 16).then_inc(s_g, 16)

        @block.vector
        def _(vector: bass.BassEngine):
            vector.wait_ge(s_x, 16)
            vector.wait_ge(s_g, 16)
            vector.tensor_sub(out=gt[:], in0=xt[:], in1=gt[:]).then_inc(s_v, 1)
            vector.wait_ge(s_mm, 1)
            vector.tensor_copy(out=res[:], in_=acc[:]).then_inc(s_res, 1)

        @block.scalar
        def _(scalar: bass.BassEngine):
            # Dummy square: prefetches the ACT function table while the
            # DMAs are still in flight (first ACTIVATE triggers the
            # load).  Reads the preamble's barrier-synced 0.0 const.
            scalar.square(out=res[:], in_=nc.const_aps.tensor(0.0, [1, 1]))
            scalar.wait_ge(s_v, 1)
            # rs[p] = sum_d ((x-c)/16)^2 = dist_p / 256
            scalar.activation(
                out=sq[:],
                in_=gt[:],
                func=mybir.ActivationFunctionType.Square,
                scale=1.0 / 16.0,
                accum_out=rs[:, :1],
            ).then_inc(s_a, 1)

        @block.tensor
        def _(tensor: bass.BassEngine):
            tensor.wait_ge(s_a, 1)
            tensor.matmul(
                out=acc[:],
                lhsT=nc.const_aps.tensor(1.0, [SHARD, 1]),
                rhs=rs[:, :1],
                start=True,
                stop=True,
            ).then_inc(s_mm, 1)

    return nc


def kernel(x: np.ndarray, label: np.ndarray, centers: np.ndarray) -> np.ndarray:
    global LAST_RESULTS
    x = np.ascontiguousarray(np.asarray(x, dtype=np.float32))
    centers = np.ascontiguousarray(np.asarray(centers, dtype=np.float32))
    lbl = np.asarray(label).astype(np.int64).ravel()
    assert x.shape == (BATCH, FEAT), x.shape
    assert centers.shape == (NUM_CLASSES, FEAT), centers.shape
    assert lbl.shape == (BATCH,), lbl.shape
    lbl32 = lbl.astype(np.int32)

    in_maps = []
    for i in range(N_CORES):
        sl = slice(i * SHARD, (i + 1) * SHARD)
        in_maps.append({"x": x[sl], "lab": lbl32[sl], "cen": centers})

    if "nc" not in _cache:
        _cache["nc"] = _build()
    res = run_bass_kernel_spmd(_cache["nc"], in_maps, core_ids=list(range(N_CORES)))
    LAST_RESULTS = res

    total = np.float64(0.0)
    for r in res.results:
        total += np.float64(r["out"][0, 0])
    return np.float32(total)
